# revision 48
# baseline (speedup 1.0000x reference)
"""Multi-head attention (B=2, S=2048, D=1024, H=16) on 8 TRN2 NeuronCores.

Sharding: tensor-parallel over heads (TP=4, 4 heads / 256 dims per core)
x data-parallel over batch (DP=2). Core c = 4*b + t handles batch b,
head group t.

Key optimizations over the straightforward schedule:
- Masked keys are compacted away on the host (the mask is per-batch and
  key-only): only ~half the keys survive, so K/V projections, scores,
  exp and AV all shrink proportionally. Padding keys in the last 128-key
  tile get an exp bias of -60 (exp ~ 1e-26, exact to fp32 eps).
- Scores are computed transposed (scores^T = [s_k, s_q]) so the exp
  output can feed AV as the stationary operand: AV runs in the
  out=[q, d] orientation (full 128-row contraction over keys, 65-wide
  output with a ones column providing softmax denominators per query
  partition). This halves AV PE time vs the [d, q] orientation.
- Softmax normalization is a per-partition reciprocal broadcast on DVE
  (denominator lives on the query partition), not a PE broadcast matmul.
- ctx[q, d] is transposed on the PE (identity matmul) so the output
  projection runs with full-width stationary operands.
- Partials are stored bf16 (halves the ReduceScatter + DMA traffic);
  b_o is added on the host after the gather.
- The Activation engine runs ONLY the exp instructions; all copies and
  normalization run on DVE; bulk loads are batched into few large DMAs
  on the SP + ACT HWDGE rings, ordered by first use (the DMA engines
  are a serial shared resource in practice).
- Every section's AV matmuls + normalize are deferred into the next
  section's filler slots (they are exp-gated; the next section's scores
  must not queue behind them on the in-order PE), and the final block
  pipelines normalize/transpose/projection per 128-query tile with the
  m0 halves pre-accumulated.

All matmul operands are bf16 (fp32 PSUM accumulation).
"""

import contextlib
import math
import numpy as np
import ml_dtypes

import concourse.bass as bass
import concourse.tile as tile
from concourse import bacc, mybir
from concourse.bass_utils import run_bass_kernel_spmd
from concourse.masks import make_identity

F32 = mybir.dt.float32
BF16 = mybir.dt.bfloat16
Exp = mybir.ActivationFunctionType.Exp
BF = ml_dtypes.bfloat16

B, S, D, H = 2, 2048, 1024, 16
DK = D // H                      # 64
TP, DP = 4, 2
HPC = H // TP                    # heads per core = 4
DSH = D // TP                    # shard dims per core = 256
QB = 512                         # query block
NQB = S // QB                    # 4
NKB = D // 128                   # 8 contraction tiles for projections
MASK_NEG = -60.0
LAG = 5                          # AV trails exp by LAG key-tiles

REPLICA_GROUPS = [[0, 1, 2, 3], [4, 5, 6, 7]]


def build_nc(nkt=9, with_collective=True):
    """nkt = number of 128-key tiles after mask compaction + padding."""
    SKP = nkt * 128
    KCH = 3                      # K-projection chunks
    KCW = SKP // KCH             # chunk width (keys); nkt*128 divisible by 3
    # fall back to per-128 chunks if not divisible
    if SKP % KCH != 0 or KCW > 512:
        KCH = nkt
        KCW = 128

    nc = bacc.Bacc("TRN2", target_bir_lowering=False, debug=False, num_devices=DP * TP)

    # ---- parameters (per-core shards, host-prepped layouts)
    xq = nc.declare_dram_parameter("xq", [NKB, 128, S], BF16, isOutput=False)
    xk = nc.declare_dram_parameter("xk", [NKB, 128, SKP], BF16, isOutput=False)
    xv = nc.declare_dram_parameter("xv", [NKB, 128, SKP], BF16, isOutput=False)
    wq = nc.declare_dram_parameter("wq", [128, NKB * DSH], BF16, isOutput=False)
    wk = nc.declare_dram_parameter("wk", [128, NKB * DSH], BF16, isOutput=False)
    wv = nc.declare_dram_parameter("wv", [128, NKB * DSH], BF16, isOutput=False)
    wo = nc.declare_dram_parameter("wo", [128, 2 * D], BF16, isOutput=False)
    bq = nc.declare_dram_parameter("bq", [128, 2], F32, isOutput=False)
    bk = nc.declare_dram_parameter("bk", [128, 2], F32, isOutput=False)
    bvb = nc.declare_dram_parameter("bvb", [128, DSH], F32, isOutput=False)
    mb = nc.declare_dram_parameter("mb", [128, nkt], F32, isOutput=False)
    out = nc.declare_dram_parameter("out", [NQB, 128, D], BF16, isOutput=True)

    with tile.TileContext(nc) as tc, contextlib.ExitStack() as ctx:
        const = ctx.enter_context(tc.tile_pool(name="const", bufs=1))
        xp = ctx.enter_context(tc.tile_pool(name="xp", bufs=1))
        proj_p = ctx.enter_context(tc.tile_pool(name="projp", bufs=2))
        vp_p = ctx.enter_context(tc.tile_pool(name="vp", bufs=nkt))
        ets_p = ctx.enter_context(tc.tile_pool(name="etsp", bufs=18))
        rec_p = ctx.enter_context(tc.tile_pool(name="recp", bufs=2))
        ctx_p = ctx.enter_context(tc.tile_pool(name="ctxp", bufs=8))
        ctxT_p = ctx.enter_context(tc.tile_pool(name="ctxTp", bufs=6))
        pos_p = ctx.enter_context(tc.tile_pool(name="posp", bufs=2))
        ps_s = ctx.enter_context(tc.tile_pool(name="pss", bufs=2, space="PSUM"))
        ps_av = ctx.enter_context(tc.tile_pool(name="psav", bufs=2, space="PSUM"))
        ps_pj = ctx.enter_context(tc.tile_pool(name="pspj", bufs=2, space="PSUM"))
        dram = ctx.enter_context(tc.tile_pool(name="dram", bufs=2, space="DRAM"))

        # ---- constants
        w_sb = {name: const.tile([128, NKB * DSH], BF16, name=f"{name}_sb")
                for name in ("wk", "wv", "wq")}
        wo_sb = const.tile([128, 2 * D], BF16)
        bq_sb = const.tile([128, 2], F32)
        bk_sb = const.tile([128, 2], F32)
        bvb_sb = const.tile([128, DSH], F32)
        mb_sb = const.tile([128, nkt], F32)
        ident = const.tile([128, 128], BF16)
        make_identity(nc, ident[:])

        # ---- persistent activation tiles
        QT = [const.tile([128, S], BF16, name=f"qt_{m}") for m in range(2)]
        KT = [const.tile([128, SKP], BF16, name=f"kt_{m}") for m in range(2)]
        VP = [vp_p.tile([128, HPC, DK + 1], BF16, name=f"vp_{st}", tag="vp")
              for st in range(nkt)]

        # ---- x input tiles (3-D tiles so loads batch into few DMAs; xk is
        #      two tiles so the first K-chain half only waits its own DMA)
        xt_ka = xp.tile([128, 4, SKP], BF16, name="x_ka", tag="xka")
        xt_kb = xp.tile([128, 4, SKP], BF16, name="x_kb", tag="xkb")
        xt_v = xp.tile([128, NKB, SKP], BF16, name="x_v", tag="xv")
        xt_q = xp.tile([128, NKB, S], BF16, name="x_q", tag="xq")

        def xk_sl(kb, csl):
            t = xt_ka if kb < 4 else xt_kb
            return t[:, kb % 4, csl]

        # ---- DMA issue order (two HWDGE rings: SP=sync, ACT=scalar).
        # The DMA engines are a shared resource; transfers run in issue
        # order, so order = priority: xk -> wq/xq(first block) -> wv/xv ->
        # xq(rest, per-qb) -> wo.
        nc.sync.dma_start(out=w_sb["wk"][:], in_=wk[:])
        nc.sync.dma_start(out=xt_ka[:], in_=xk[0:4].rearrange("k p s -> p k s"))
        nc.scalar.dma_start(out=xt_kb[:], in_=xk[4:NKB].rearrange("k p s -> p k s"))
        nc.scalar.dma_start(out=bk_sb[:], in_=bk[:])
        nc.scalar.dma_start(out=bq_sb[:], in_=bq[:])
        nc.scalar.dma_start(out=mb_sb[:], in_=mb[:])
        nc.sync.dma_start(out=w_sb["wq"][:], in_=wq[:])
        nc.sync.dma_start(out=xt_q[:, :, 0:QB],
                          in_=xq[:, :, 0:QB].rearrange("k p s -> p k s"))
        nc.scalar.dma_start(out=w_sb["wv"][:], in_=wv[:])
        nc.scalar.dma_start(out=bvb_sb[:], in_=bvb[:])
        nc.scalar.dma_start(out=xt_v[:], in_=xv[:].rearrange("k p s -> p k s"))
        for qx in range(1, NQB):
            nc.sync.dma_start(
                out=xt_q[:, :, qx * QB:(qx + 1) * QB],
                in_=xq[:, :, qx * QB:(qx + 1) * QB].rearrange("k p s -> p k s"))
        nc.scalar.dma_start(out=wo_sb[:], in_=wo[:])

        # ---- projections (split into two emission halves to limit PE
        #      head-of-line blocking; half 0 allocates psum + runs kb 0..3,
        #      half 1 runs kb 4..7 and the bias-add copy)
        _chain_ps = {}

        def k_chain(m, ch, half=None):
            halves = [0, 1] if half is None else [half]
            for hf in halves:
                if hf == 0:
                    _chain_ps[("k", m, ch)] = ps_pj.tile(
                        [128, 512], F32, name=f"ps_k_{m}_{ch}", tag="pj")
                ps = _chain_ps[("k", m, ch)]
                for kb in range(hf * 4, hf * 4 + 4):
                    nc.tensor.matmul(
                        ps[:, 0:KCW],
                        w_sb["wk"][:, kb * DSH + m * 128: kb * DSH + (m + 1) * 128],
                        xk_sl(kb, slice(ch * KCW, (ch + 1) * KCW)),
                        start=(kb == 0), stop=(kb == NKB - 1),
                    )
                if hf == 1:
                    nc.vector.tensor_scalar_add(
                        KT[m][:, ch * KCW:(ch + 1) * KCW], ps[:, 0:KCW],
                        bk_sb[:, m:m + 1])

        def q_chain(m, qb, half=None):
            halves = [0, 1] if half is None else [half]
            for hf in halves:
                if hf == 0:
                    _chain_ps[("q", m, qb)] = ps_pj.tile(
                        [128, 512], F32, name=f"ps_q_{m}_{qb}", tag="pj")
                ps = _chain_ps[("q", m, qb)]
                for kb in range(hf * 4, hf * 4 + 4):
                    nc.tensor.matmul(
                        ps[:],
                        w_sb["wq"][:, kb * DSH + m * 128: kb * DSH + (m + 1) * 128],
                        xt_q[:, kb, qb * QB:(qb + 1) * QB],
                        start=(kb == 0), stop=(kb == NKB - 1),
                    )
                if hf == 1:
                    nc.vector.tensor_scalar_add(
                        QT[m][:, qb * QB:(qb + 1) * QB], ps[:], bq_sb[:, m:m + 1])

        def v_chain(st):
            ps = ps_pj.tile([128, 512], F32, name=f"ps_v_{st}", tag="pj")
            for kb in range(NKB):
                nc.tensor.matmul(
                    ps[:, 0:DSH],
                    xt_v[:, kb, st * 128:(st + 1) * 128],
                    w_sb["wv"][:, kb * DSH:(kb + 1) * DSH],
                    start=(kb == 0), stop=(kb == NKB - 1),
                )
            vp = VP[st]
            nc.gpsimd.memset(vp[:, :, DK:DK + 1], 1.0)
            ps3 = ps[:, 0:DSH].rearrange("p (h d) -> p h d", h=HPC)
            bv3 = bvb_sb.rearrange("p (h d) -> p h d", h=HPC)
            nc.vector.tensor_add(vp[:, :, 0:DK], ps3, bv3)

        # ---- phase B helpers
        def emit_av(av, ets, kt, m):
            for h in range(2):
                for qt in range(4):
                    nc.tensor.matmul(
                        av[h][:, qt, :],
                        ets[:, h * QB + qt * 128: h * QB + (qt + 1) * 128],
                        VP[kt][:, 2 * m + h, :],
                        start=(kt == 0 and qt == 0),
                        stop=(kt == nkt - 1 and qt == 3),
                        skip_group_check=True,
                    )

        ctxT = {}
        partials = {}

        def emit_outproj_item(qbx, qt, dh):
            pso = ps_pj.tile([128, 512], F32, name=f"pso_{qbx}_{qt}_{dh}", tag="pj")
            for mm in range(2):
                nc.tensor.matmul(
                    pso[:],
                    ctxT[(qbx, mm)][:, qt, :],
                    wo_sb[:, mm * D + dh * QB: mm * D + (dh + 1) * QB],
                    start=(mm == 0), stop=(mm == 1),
                )
            nc.vector.tensor_copy(
                pos_sb[qbx][:, qt, dh * QB:(dh + 1) * QB], pso[:])

        def emit_partial_dma(qbx, qt):
            partial = partials[qbx]
            nc.sync.dma_start(
                out=partial[qt * 128:(qt + 1) * 128, :],
                in_=pos_sb[qbx][:, qt, :])

        _op_pairs = {}

        def op_pair(qt, mms):
            """Final-qb output projection for one 128-query tile, both
            dout halves in a single [128, 1024] psum tile from the scores
            pool (free once the last exp has read its slot)."""
            if 0 in mms:
                _op_pairs[qt] = ps_s.tile(
                    [128, 2 * QB], F32, name=f"opp_{qt}", tag="pss")
            pp = _op_pairs[qt]
            for mm in mms:
                for dh in range(2):
                    nc.tensor.matmul(
                        pp[:, dh * QB:(dh + 1) * QB],
                        ctxT[(NQB - 1, mm)][:, qt, :],
                        wo_sb[:, mm * D + dh * QB: mm * D + (dh + 1) * QB],
                        start=(mm == 0), stop=(mm == 1))
            if 1 in mms:
                # ACT is idle after its last exp: run the tail's psum->
                # sbuf copies there so they overlap DVE's normalize work
                nc.scalar.copy(pos_sb[NQB - 1][:, qt, :], pp[:])
                emit_partial_dma(NQB - 1, qt)

        def emit_rs(qbx):
            partial = partials[qbx]
            rs_out = dram.tile([128, D], BF16, name=f"rs_{qbx}", tag="rs")
            if with_collective:
                nc.gpsimd.collective_compute(
                    "ReduceScatter", mybir.AluOpType.add,
                    replica_groups=REPLICA_GROUPS,
                    ins=[partial[:].opt()], outs=[rs_out[:].opt()])
            else:
                nc.sync.dma_start(out=rs_out[:], in_=partial[0:128, :])
            nc.sync.dma_start(out=out[qbx], in_=rs_out[:])

        def emit_transposes(qbx, mx, qts=None):
            """Transpose ctx[q, (qt,d)] -> ctxT[d, qt, q].

            Default path: one xbar DMA-transpose (off the PE/DVE). The
            final block passes explicit qts to pipeline per-qt through
            the PE (lower latency on the critical tail).
            """
            if (qbx, mx) not in ctxT:
                ctxT[(qbx, mx)] = ctxT_p.tile(
                    [128, 4, 128], BF16, name=f"ctxT_{qbx}_{mx}", tag="ctxT")
            if qts is None:
                nc.sync.dma_start_transpose(
                    out=ctxT[(qbx, mx)][:], in_=ctx_sbs[(qbx, mx)][:])
                return
            if (qbx, mx) not in _trs:
                _trs[(qbx, mx)] = ps_pj.tile(
                    [128, 4, 128], BF16, name=f"tr_{qbx}_{mx}", tag="pj")
            tr = _trs[(qbx, mx)]
            for qt in qts:
                nc.tensor.matmul(
                    tr[:, qt, :], ctx_sbs[(qbx, mx)][:, qt, :], ident[:],
                    start=(qt == 0), stop=(qt == 3),
                    is_transpose=True, skip_group_check=True)
                nc.vector.tensor_copy(ctxT[(qbx, mx)][:, qt, :], tr[:, qt, :])

        def emit_avtail(qb, m, kts):
            if kts.start == 0:
                avs[(qb, m)] = [
                    ps_av.tile([128, 4, DK + 1], F32,
                               name=f"av_{qb}_{m}_{h}", tag="av")
                    for h in range(2)]
            for kt2 in kts:
                emit_av(avs[(qb, m)], ets_store[(qb, m)][kt2], kt2, m)

        def emit_norm(qb, m, qts):
            av = avs[(qb, m)]
            if (qb, m) not in recs:
                recs[(qb, m)] = rec_p.tile(
                    [128, 2, 4], F32, name=f"rec_{qb}_{m}", tag="rec")
                for h in range(2):
                    nc.vector.reciprocal(recs[(qb, m)][:, h, :], av[h][:, :, DK])
                ctx_sbs[(qb, m)] = ctx_p.tile(
                    [128, 4, 128], BF16, name=f"ctx_{qb}_{m}", tag="ctx")
            rec = recs[(qb, m)]
            for qt in qts:
                for h in range(2):
                    nc.vector.tensor_scalar_mul(
                        ctx_sbs[(qb, m)][:, qt, h * DK:(h + 1) * DK],
                        av[h][:, qt, 0:DK],
                        rec[:, h, qt:qt + 1])

        # ---- phase A emit: only what gates the first scores; the rest is
        #      paced through phase B's exp loop as PE fillers. Halves are
        #      emitted separately so kb0-3 only wait the first xk DMA.
        k_chain(0, 0, 0)
        k_chain(0, 0, 1)
        q_chain(0, 0, 0)
        q_chain(0, 0, 1)

        pos_sb = {}
        ctx_sbs = {}
        avs = {}
        recs = {}
        ets_store = {}
        _trs = {}
        carry = []        # deferred work from the previous section
        # ---- phase B
        for qb in range(NQB):
            pos_sb[qb] = pos_p.tile([128, NQB, D], BF16, name=f"pos_{qb}", tag="pos")
            partials[qb] = dram.tile([QB, D], BF16, name=f"partial_{qb}", tag="partial")
            for m in range(2):
                # fillers: previous section's deferred AV+normalize first
                # (one small chunk per exp step so scores stay ahead of
                # the ACT stream), then this section's own prep work
                prev_carry = list(carry)
                carry = []
                fillers = list(prev_carry)
                if qb == 0 and m == 0:
                    fillers += [lambda: k_chain(0, 1, 0), lambda: k_chain(0, 1, 1),
                                lambda: q_chain(1, 0, 0), lambda: q_chain(1, 0, 1),
                                lambda: k_chain(0, 2, 0), lambda: k_chain(0, 2, 1),
                                lambda: k_chain(1, 0, 0), lambda: k_chain(1, 0, 1)]
                elif qb == 0 and m == 1:
                    # own K/V chains FIRST: section (0,0)'s deferred AVs
                    # (prev_carry) need the V tiles, so they follow them
                    fillers = [lambda: k_chain(1, 1, 0), lambda: k_chain(1, 1, 1),
                               lambda: k_chain(1, 2, 0), lambda: k_chain(1, 2, 1)]
                    fillers += [lambda st=st: v_chain(st) for st in range(nkt)]
                    fillers += [lambda: q_chain(0, 1, 0), lambda: q_chain(0, 1, 1),
                                lambda: q_chain(1, 1, 0), lambda: q_chain(1, 1, 1)]
                    fillers += prev_carry
                    fillers += [lambda: emit_transposes(0, 0)]
                elif m == 1:
                    fillers += [lambda: emit_transposes(qb, 0)]
                    if qb + 1 < NQB:
                        fillers += [lambda: q_chain(0, qb + 1, 0),
                                    lambda: q_chain(0, qb + 1, 1),
                                    lambda: q_chain(1, qb + 1, 0),
                                    lambda: q_chain(1, qb + 1, 1)]
                    else:
                        # pre-accumulate the m0 half of the last output
                        # projection for two query tiles; padded past the
                        # slot count so the psum allocs follow the last
                        # scores tile (shared ring)
                        fillers += [None] * (nkt - len(fillers))
                        fillers += [lambda: op_pair(0, [0]), lambda: op_pair(1, [0])]
                else:  # m == 0, qb > 0
                    fillers += [lambda: emit_transposes(qb - 1, 1)]
                    for kt in range(8):
                        fillers += [(lambda qt=kt // 2, dh=kt % 2:
                                     emit_outproj_item(qb - 1, qt, dh))]
                        if kt % 2 == 1:
                            fillers += [(lambda qt=kt // 2:
                                         emit_partial_dma(qb - 1, qt))]
                    fillers += [lambda: emit_rs(qb - 1)]
                fid = 0
                etss = []
                ets_store[(qb, m)] = etss
                for kt in range(nkt):
                    pss = ps_s.tile([128, 2 * QB], F32, name=f"pss_{qb}_{m}_{kt}", tag="pss")
                    for h in range(2):
                        nc.tensor.matmul(
                            pss[:, h * QB:(h + 1) * QB],
                            KT[m][h * 64:(h + 1) * 64, kt * 128:(kt + 1) * 128],
                            QT[m][h * 64:(h + 1) * 64, qb * QB:(qb + 1) * QB],
                            start=True, stop=True)
                    et = ets_p.tile([128, 2 * QB], BF16, name=f"ets_{qb}_{m}_{kt}", tag="ets")
                    if qb == NQB - 1 and m == 1 and kt >= nkt - 2:
                        # split the very last exps per head so the tail's
                        # AV + reciprocal chain can start half a tile early
                        for h in range(2):
                            nc.scalar.activation(
                                et[:, h * QB:(h + 1) * QB],
                                pss[:, h * QB:(h + 1) * QB], Exp,
                                bias=mb_sb[:, kt:kt + 1],
                                scale=1.0 / math.sqrt(DK))
                    else:
                        nc.scalar.activation(et[:], pss[:], Exp,
                                             bias=mb_sb[:, kt:kt + 1],
                                             scale=1.0 / math.sqrt(DK))
                    etss.append(et)
                    if fid < len(fillers):
                        if fillers[fid] is not None:
                            fillers[fid]()
                        fid += 1
                while fid < len(fillers):
                    if fillers[fid] is not None:
                        fillers[fid]()
                    fid += 1
                # defer ALL AV matmuls + normalize into the next section's
                # filler slots: they are exp-gated, and the next section's
                # scores must not sit behind them in the PE queue. (The
                # first section's land in (0,1)'s leftovers, after the
                # V tiles they need.)
                avchunks = [range(k, min(k + 2, nkt)) for k in range(0, nkt, 2)]
                carry += [(lambda qb=qb, m=m, r=r: emit_avtail(qb, m, r))
                          for r in avchunks]
                carry += [
                    (lambda qb=qb, m=m: emit_norm(qb, m, range(0, 2))),
                    (lambda qb=qb, m=m: emit_norm(qb, m, range(2, 4))),
                ]

        # final block: AV chunks (exp-gated), then a per-qt pipeline of
        # normalize (DVE) -> transpose (PE) -> output projection (PE) ->
        # partial copy (GPSIMD) -> partial DMA, so the three engines
        # overlap on the tail. qt 0/1 had their m0 halves pre-accumulated.
        for fn in carry[:-2]:
            fn()
        for qt in range(4):
            emit_norm(NQB - 1, 1, range(qt, qt + 1))
            emit_transposes(NQB - 1, 1, qts=[qt])
            op_pair(qt, [1] if qt < 2 else [0, 1])
        emit_rs(NQB - 1)

    nc.compile()
    return nc


def _prep_inputs(q_in, k_in, v_in, mask, w_q, b_q, w_k, b_k, w_v, b_v, w_o, b_o):
    keep_b = [np.nonzero(mask[b, 0, 0, :])[0] for b in range(B)]
    nkt = max(1, max((len(k) + 127) // 128 for k in keep_b))
    SKP = nkt * 128

    xq_b, xk_b, xv_b, mb_b = [], [], [], []
    for b in range(B):
        keep = keep_b[b]
        nk = len(keep)
        xq_b.append(np.ascontiguousarray(q_in[b].T).astype(BF).reshape(NKB, 128, S))
        xkf = np.zeros((D, SKP), np.float32)
        xkf[:, :nk] = k_in[b][keep].T
        xk_b.append(xkf.astype(BF).reshape(NKB, 128, SKP))
        xvf = np.zeros((D, SKP), np.float32)
        xvf[:, :nk] = v_in[b][keep].T
        xv_b.append(xvf.astype(BF).reshape(NKB, 128, SKP))
        mbias = np.zeros(SKP, np.float32)
        mbias[nk:] = MASK_NEG
        mb_b.append(np.ascontiguousarray(mbias.reshape(nkt, 128).T))

    in_maps = []
    for c in range(DP * TP):
        b, t = c // TP, c % TP
        sl = slice(DSH * t, DSH * (t + 1))

        def pack_w(w_t, nblk):
            cols = w_t.shape[1]
            return np.ascontiguousarray(
                w_t.reshape(nblk, 128, cols).transpose(1, 0, 2).reshape(128, nblk * cols)
            ).astype(BF)

        in_maps.append({
            "xq": xq_b[b], "xk": xk_b[b], "xv": xv_b[b],
            "wq": pack_w(np.ascontiguousarray(w_q[sl, :].T), NKB),
            "wk": pack_w(np.ascontiguousarray(w_k[sl, :].T), NKB),
            "wv": pack_w(np.ascontiguousarray(w_v[sl, :].T), NKB),
            "wo": pack_w(np.ascontiguousarray(w_o[:, sl].T), 2),
            "bq": np.ascontiguousarray(b_q[sl].astype(np.float32).reshape(2, 128).T),
            "bk": np.ascontiguousarray(b_k[sl].astype(np.float32).reshape(2, 128).T),
            "bvb": np.ascontiguousarray(
                np.broadcast_to(b_v[sl].astype(np.float32), (128, DSH))),
            "mb": mb_b[b],
        })
    return in_maps, nkt


_NC_CACHE = {}


def kernel(q_in, k_in, v_in, mask, w_q, b_q, w_k, b_k, w_v, b_v, w_o, b_o):
    q_in, k_in, v_in, mask = (np.asarray(a) for a in (q_in, k_in, v_in, mask))
    w_q, b_q, w_k, b_k = (np.asarray(a) for a in (w_q, b_q, w_k, b_k))
    w_v, b_v, w_o, b_o = (np.asarray(a) for a in (w_v, b_v, w_o, b_o))
    in_maps, nkt = _prep_inputs(q_in, k_in, v_in, mask,
                                w_q, b_q, w_k, b_k, w_v, b_v, w_o, b_o)
    if nkt not in _NC_CACHE:
        _NC_CACHE[nkt] = build_nc(nkt=nkt)
        _NC_CACHE["nkt"] = nkt
    nc = _NC_CACHE[nkt]
    res = run_bass_kernel_spmd(nc, in_maps, list(range(DP * TP))).results
    bo32 = b_o.astype(np.float32)
    full = np.empty((B, S, D), np.float32)
    for b in range(B):
        for r in range(TP):
            o = res[TP * b + r]["out"]          # [NQB, 128, D] bf16
            for qb in range(NQB):
                row = qb * QB + r * 128
                full[b, row:row + 128] = o[qb].astype(np.float32) + bo32
    return full
